# revision 6
# baseline (speedup 1.0000x reference)
"""MLA (DeepSeek-style) attention block on 8 Trainium2 NeuronCores.

Sharding: token-parallel LoRA-A path (8x512 tokens) -> AllGather(latents)
-> head-parallel up-proj + causal attention (2 heads x 2 batches / core)
-> AllToAll(attention out) -> token-parallel output projection.

All activations/weights bf16 on the tensor engine (fp32 PSUM accumulation),
fp32 softmax statistics, fp32 output.
"""
import sys

sys.path.insert(0, "/opt/trn_rl_repo")

import numpy as np
import ml_dtypes

import concourse.bacc as bacc
import concourse.mybir as mybir
import concourse.tile as tile
from concourse.bass_utils import run_bass_kernel_spmd

# ---- problem sizes (hardcoded per spec) ----
HID = 2048; H = 16; QLR = 1536; KVLR = 512
DN = 128; DR = 64; DV = 128; DQ = DN + DR
B = 2; S = 2048
THETA = 10000.0; EPS = 1e-6

NCORES = 8
T = B * S              # 4096 flattened tokens
TPC = T // NCORES      # 512 tokens per core
HPC = H // NCORES      # 2 heads per core
P = 128
NHID = HID // P        # 16
NQLR = QLR // P        # 12
NKVA = (KVLR + DR + P - 1) // P  # 5 blocks (4 full + 64)
LAT = QLR + KVLR + DR  # 2112 latent dims
QT_PER_B = S // 512    # 4 q-tiles of 512 per (b,h) unit
KB_PER_B = S // P      # 16 k-chunks of 128 per batch

BF16 = mybir.dt.bfloat16
F32 = mybir.dt.float32
AF = mybir.ActivationFunctionType

_NC_CACHE = None


def build_nc():
    nc = bacc.Bacc(None, target_bir_lowering=False, debug=False, num_devices=NCORES)

    # ---- per-core external inputs ----
    hidT = nc.dram_tensor("hidT", [HID, TPC], BF16, kind="ExternalInput")
    wqaT = nc.dram_tensor("wqaT", [HID, QLR], BF16, kind="ExternalInput")
    wkvaT = nc.dram_tensor("wkvaT", [HID, KVLR + DR], BF16, kind="ExternalInput")
    wqbT = nc.dram_tensor("wqbT", [QLR, HPC * DQ], BF16, kind="ExternalInput")
    wkvbkT = nc.dram_tensor("wkvbkT", [KVLR, HPC * DN], BF16, kind="ExternalInput")
    wkvbvT = nc.dram_tensor("wkvbvT", [KVLR, HPC * DV], BF16, kind="ExternalInput")
    woT = nc.dram_tensor("woT", [H * DV, HID], BF16, kind="ExternalInput")
    cosq = nc.dram_tensor("cosq", [P, T], F32, kind="ExternalInput")
    sinq = nc.dram_tensor("sinq", [P, T], F32, kind="ExternalInput")
    cosl = nc.dram_tensor("cosl", [DR, TPC], F32, kind="ExternalInput")
    sinl = nc.dram_tensor("sinl", [DR, TPC], F32, kind="ExternalInput")
    masks = nc.dram_tensor("masks", [P, 4 * 512], BF16, kind="ExternalInput")
    outT = nc.dram_tensor("outT", [HID, TPC], F32, kind="ExternalOutput")

    with tile.TileContext(nc) as tc:
        with tc.tile_pool(name="dram", bufs=1, space="DRAM") as dram, \
             tc.tile_pool(name="const", bufs=1) as const:
            lat_in = dram.tile([LAT, TPC], BF16)
            lat_all = dram.tile([NCORES * LAT, TPC], BF16, addr_space="Shared")
            a2a_in = dram.tile([H * DV, TPC], BF16)
            a2a_out = dram.tile([H * DV, TPC], BF16)

            ones_col = const.tile([P, 1], BF16)
            nc.vector.memset(ones_col[:], 1.0)
            ones_row = const.tile([1, P], F32)
            nc.vector.memset(ones_row[:], 1.0)
            eps_t = const.tile([1, 1], F32)
            nc.vector.memset(eps_t[:], EPS)

            # ================= Phase 1: token-parallel LoRA-A =================
            with tc.tile_pool(name="p1w", bufs=1) as p1w, \
                 tc.tile_pool(name="p1a", bufs=1) as p1a, \
                 tc.tile_pool(name="p1t", bufs=3) as p1t, \
                 tc.tile_pool(name="p1n", bufs=1) as p1n, \
                 tc.tile_pool(name="ps1", bufs=3, space="PSUM") as ps1, \
                 tc.tile_pool(name="ps1s", bufs=1, space="PSUM") as ps1s, \
                 tc.tile_pool(name="ps1b", bufs=2, space="PSUM") as ps1b:
                hid_sb = p1a.tile([P, NHID * TPC], BF16)
                for kc in range(NHID):
                    nc.sync.dma_start(hid_sb[:, kc * TPC:(kc + 1) * TPC],
                                      hidT.ap()[kc * P:(kc + 1) * P, :])
                wqa_sb = p1w.tile([P, NHID * QLR], BF16)
                for kc in range(NHID):
                    nc.sync.dma_start(wqa_sb[:, kc * QLR:(kc + 1) * QLR],
                                      wqaT.ap()[kc * P:(kc + 1) * P, :])
                wkva_sb = p1w.tile([P, NHID * (KVLR + DR)], BF16)
                for kc in range(NHID):
                    nc.sync.dma_start(wkva_sb[:, kc * (KVLR + DR):(kc + 1) * (KVLR + DR)],
                                      wkvaT.ap()[kc * P:(kc + 1) * P, :])

                # --- cq = hidden @ w_qa.T (d-major) + ssq for rmsnorm ---
                cq_f32 = p1a.tile([P, NQLR * TPC], F32)
                ssq_q = ps1s.tile([1, TPC], F32)
                for m in range(NQLR):
                    ps = ps1.tile([P, TPC], F32, tag="proj")
                    for kc in range(NHID):
                        nc.tensor.matmul(ps[:], wqa_sb[:, kc * QLR + m * P: kc * QLR + (m + 1) * P],
                                         hid_sb[:, kc * TPC:(kc + 1) * TPC],
                                         start=(kc == 0), stop=(kc == NHID - 1))
                    nc.scalar.copy(cq_f32[:, m * TPC:(m + 1) * TPC], ps[:])
                    sq = p1t.tile([P, TPC], BF16, tag="sq")
                    nc.vector.tensor_mul(sq[:], cq_f32[:, m * TPC:(m + 1) * TPC],
                                         cq_f32[:, m * TPC:(m + 1) * TPC])
                    nc.tensor.matmul(ssq_q[:], ones_col[:], sq[:],
                                     start=(m == 0), stop=(m == NQLR - 1),
                                     skip_group_check=True)

                # --- ckv joint (d-major), blocks 0-3 normed, block 4 = k_pe ---
                CKW = KVLR + DR
                ckv_f32 = p1a.tile([P, 4 * TPC], F32)
                ssq_kv = ps1s.tile([1, TPC], F32)
                for m in range(4):
                    ps = ps1.tile([P, TPC], F32, tag="proj")
                    for kc in range(NHID):
                        nc.tensor.matmul(ps[:], wkva_sb[:, kc * CKW + m * P: kc * CKW + (m + 1) * P],
                                         hid_sb[:, kc * TPC:(kc + 1) * TPC],
                                         start=(kc == 0), stop=(kc == NHID - 1))
                    nc.scalar.copy(ckv_f32[:, m * TPC:(m + 1) * TPC], ps[:])
                    sq = p1t.tile([P, TPC], BF16, tag="sq")
                    nc.vector.tensor_mul(sq[:], ckv_f32[:, m * TPC:(m + 1) * TPC],
                                         ckv_f32[:, m * TPC:(m + 1) * TPC])
                    nc.tensor.matmul(ssq_kv[:], ones_col[:], sq[:],
                                     start=(m == 0), stop=(m == 3),
                                     skip_group_check=True)

                # k_pe block [64, TPC] + rope
                ps_pe = ps1.tile([DR, TPC], F32, tag="proj")
                for kc in range(NHID):
                    nc.tensor.matmul(ps_pe[:], wkva_sb[:, kc * CKW + KVLR: kc * CKW + KVLR + DR],
                                     hid_sb[:, kc * TPC:(kc + 1) * TPC],
                                     start=(kc == 0), stop=(kc == NHID - 1))
                cos_sb = p1n.tile([DR, TPC], F32)
                sin_sb = p1n.tile([DR, TPC], F32)
                nc.sync.dma_start(cos_sb[:], cosl.ap()[:])
                nc.sync.dma_start(sin_sb[:], sinl.ap()[:])
                rot = p1t.tile([DR, TPC], F32, tag="rot")
                HDR = DR // 2
                nc.scalar.mul(rot[0:HDR, :], ps_pe[HDR:DR, :], -1.0)
                nc.scalar.copy(rot[HDR:DR, :], ps_pe[0:HDR, :])
                t1 = p1t.tile([DR, TPC], F32, tag="t1")
                nc.vector.tensor_mul(t1[:], ps_pe[:], cos_sb[:])
                nc.vector.tensor_mul(rot[:], rot[:], sin_sb[:])
                pe_out = p1t.tile([DR, TPC], BF16, tag="peo")
                nc.vector.tensor_add(pe_out[:], t1[:], rot[:])
                nc.sync.dma_start(lat_in[QLR + KVLR:LAT, :], pe_out[:])

                # --- rmsnorm scales: rnorm = 1/sqrt(mean sq + eps) ---
                sq_norm = p1n.tile([1, TPC], F32, tag="nrmq")
                nc.scalar.activation(sq_norm[:], ssq_q[:], AF.Sqrt, bias=eps_t[:], scale=1.0 / QLR)
                rn_q = p1n.tile([1, TPC], F32, tag="rnq")
                nc.vector.reciprocal(rn_q[:], sq_norm[:])
                kv_norm = p1n.tile([1, TPC], F32, tag="nrmk")
                nc.scalar.activation(kv_norm[:], ssq_kv[:], AF.Sqrt, bias=eps_t[:], scale=1.0 / KVLR)
                rn_kv = p1n.tile([1, TPC], F32, tag="rnk")
                nc.vector.reciprocal(rn_kv[:], kv_norm[:])

                bq = ps1b.tile([P, TPC], F32, tag="bc")
                nc.tensor.matmul(bq[:], ones_row[:], rn_q[:], start=True, stop=True)
                for m in range(NQLR):
                    lat_sb = p1t.tile([P, TPC], BF16, tag="lat")
                    nc.vector.tensor_mul(lat_sb[:], cq_f32[:, m * TPC:(m + 1) * TPC], bq[:])
                    nc.sync.dma_start(lat_in[m * P:(m + 1) * P, :], lat_sb[:])
                bkv = ps1b.tile([P, TPC], F32, tag="bc")
                nc.tensor.matmul(bkv[:], ones_row[:], rn_kv[:], start=True, stop=True)
                for m in range(4):
                    lat_sb = p1t.tile([P, TPC], BF16, tag="lat")
                    nc.vector.tensor_mul(lat_sb[:], ckv_f32[:, m * TPC:(m + 1) * TPC], bkv[:])
                    nc.sync.dma_start(lat_in[QLR + m * P: QLR + (m + 1) * P, :], lat_sb[:])

            # ================= AllGather latents =================
            nc.gpsimd.collective_compute(
                "AllGather", mybir.AluOpType.bypass,
                replica_groups=[list(range(NCORES))],
                ins=[lat_in.opt()], outs=[lat_all.opt()],
            )

            # ================= Phase 2: head-parallel up-proj =================
            # persistent attention operands
            with tc.tile_pool(name="att_a", bufs=1) as att_a:
                qnope = att_a.tile([P, 2 * T], BF16)     # [h][128, T]
                qpe = att_a.tile([P, T], BF16)           # rows 0-63 h0, 64-127 h1
                knope = att_a.tile([P, 2 * T], BF16)
                kpe2 = att_a.tile([P, T], BF16)          # duplicated rows (h0:0-63, h1:64-127)
                v_sb = att_a.tile([P, (T // P) * (HPC * DV)], BF16)  # token-major

                with tc.tile_pool(name="p2w", bufs=1) as p2w, \
                     tc.tile_pool(name="p2a", bufs=2) as p2a, \
                     tc.tile_pool(name="p2t", bufs=3) as p2t, \
                     tc.tile_pool(name="ps2", bufs=4, space="PSUM") as ps2:
                    wqb_sb = p2w.tile([P, NQLR * (HPC * DQ)], BF16)
                    WQB = HPC * DQ  # 384
                    for kc in range(NQLR):
                        nc.sync.dma_start(wqb_sb[:, kc * WQB:(kc + 1) * WQB],
                                          wqbT.ap()[kc * P:(kc + 1) * P, :])
                    wkk_sb = p2w.tile([P, 4 * (HPC * DN)], BF16)
                    wkv_sb = p2w.tile([P, 4 * (HPC * DV)], BF16)
                    WKK = HPC * DN  # 256
                    for kc in range(4):
                        nc.sync.dma_start(wkk_sb[:, kc * WKK:(kc + 1) * WKK],
                                          wkvbkT.ap()[kc * P:(kc + 1) * P, :])
                        nc.sync.dma_start(wkv_sb[:, kc * WKK:(kc + 1) * WKK],
                                          wkvbvT.ap()[kc * P:(kc + 1) * P, :])
                    cosq_sb = p2w.tile([P, T], F32)
                    sinq_sb = p2w.tile([P, T], F32)
                    nc.sync.dma_start(cosq_sb[:], cosq.ap()[:])
                    nc.sync.dma_start(sinq_sb[:], sinq.ap()[:])

                    for j in range(NCORES):  # gathered token blocks of TPC
                        base = j * LAT
                        cq_j = p2a.tile([P, NQLR * TPC], BF16, tag="cqj")
                        for r in range(NQLR):
                            nc.sync.dma_start(cq_j[:, r * TPC:(r + 1) * TPC],
                                              lat_all[base + r * P: base + (r + 1) * P, :])
                        ckv_j = p2a.tile([P, 4 * TPC], BF16, tag="ckvj")
                        for r in range(4):
                            nc.sync.dma_start(ckv_j[:, r * TPC:(r + 1) * TPC],
                                              lat_all[base + QLR + r * P: base + QLR + (r + 1) * P, :])
                        # k_pe rows duplicated into both halves
                        nc.sync.dma_start(kpe2[0:DR, j * TPC:(j + 1) * TPC],
                                          lat_all[base + QLR + KVLR: base + LAT, :])
                        nc.sync.dma_start(kpe2[DR:P, j * TPC:(j + 1) * TPC],
                                          lat_all[base + QLR + KVLR: base + LAT, :])

                        # q nope blocks (m=0,1) and pe block (m=2)
                        for m in range(3):
                            ps = ps2.tile([P, TPC], F32, tag="proj")
                            for kc in range(NQLR):
                                nc.tensor.matmul(
                                    ps[:], wqb_sb[:, kc * WQB + m * P: kc * WQB + (m + 1) * P],
                                    cq_j[:, kc * TPC:(kc + 1) * TPC],
                                    start=(kc == 0), stop=(kc == NQLR - 1))
                            if m < 2:
                                nc.scalar.copy(qnope[:, m * T + j * TPC: m * T + (j + 1) * TPC], ps[:])
                            else:
                                cs = cosq_sb[:, j * TPC:(j + 1) * TPC]
                                sn = sinq_sb[:, j * TPC:(j + 1) * TPC]
                                rot = p2t.tile([P, TPC], F32, tag="rot")
                                HDR = DR // 2
                                for g in range(2):  # two 64-row head groups
                                    o = g * DR
                                    nc.scalar.mul(rot[o:o + HDR, :], ps[o + HDR:o + DR, :], -1.0)
                                    nc.scalar.copy(rot[o + HDR:o + DR, :], ps[o:o + HDR, :])
                                t1 = p2t.tile([P, TPC], F32, tag="t1")
                                nc.vector.tensor_mul(t1[:], ps[:], cs)
                                nc.vector.tensor_mul(rot[:], rot[:], sn)
                                nc.vector.tensor_add(qpe[:, j * TPC:(j + 1) * TPC], t1[:], rot[:])

                        # k nope blocks
                        for m in range(HPC):
                            ps = ps2.tile([P, TPC], F32, tag="proj")
                            for kc in range(4):
                                nc.tensor.matmul(
                                    ps[:], wkk_sb[:, kc * WKK + m * P: kc * WKK + (m + 1) * P],
                                    ckv_j[:, kc * TPC:(kc + 1) * TPC],
                                    start=(kc == 0), stop=(kc == 3))
                            nc.scalar.copy(knope[:, m * T + j * TPC: m * T + (j + 1) * TPC], ps[:])

                        # v token-major: [tok 128, 2*DV]
                        for tb in range(TPC // P):
                            ps = ps2.tile([P, HPC * DV], F32, tag="proj")
                            for kc in range(4):
                                nc.tensor.matmul(
                                    ps[:], ckv_j[:, kc * TPC + tb * P: kc * TPC + (tb + 1) * P],
                                    wkv_sb[:, kc * WKK:(kc + 1) * WKK],
                                    start=(kc == 0), stop=(kc == 3))
                            jb = j * (TPC // P) + tb
                            nc.scalar.copy(v_sb[:, jb * WKK:(jb + 1) * WKK], ps[:])

                # ================= attention (4 causal units) =================
                with tc.tile_pool(name="attc", bufs=1) as attc, \
                     tc.tile_pool(name="att_t", bufs=4) as att_t, \
                     tc.tile_pool(name="att_o", bufs=1) as att_o, \
                     tc.tile_pool(name="ps_s", bufs=3, space="PSUM") as ps_s_pool, \
                     tc.tile_pool(name="ps_o", bufs=2, space="PSUM") as ps_o_pool, \
                     tc.tile_pool(name="ps_d", bufs=2, space="PSUM") as ps_d_pool:
                    mask_sb = attc.tile([P, 4 * 512], BF16)
                    nc.sync.dma_start(mask_sb[:], masks.ap()[:])

                    for u in range(4):  # unit = (h_local, b)
                        hl, bb = u % 2, u // 2
                        for qt in range(QT_PER_B):
                            qoff = bb * S + qt * 512
                            ps_o = ps_o_pool.tile([P, 512], F32, tag="pso")
                            ps_d = ps_d_pool.tile([1, 512], F32, tag="psd")
                            nkc = 4 * (qt + 1)
                            for kc in range(nkc):
                                koff = bb * S + kc * P
                                ps_sc = ps_s_pool.tile([P, 512], F32, tag="pss")
                                nc.tensor.matmul(
                                    ps_sc[:], knope[:, hl * T + koff: hl * T + koff + P],
                                    qnope[:, hl * T + qoff: hl * T + qoff + 512],
                                    start=True, stop=False)
                                nc.tensor.matmul(
                                    ps_sc[:], kpe2[hl * DR: hl * DR + DR, koff: koff + P],
                                    qpe[hl * DR: hl * DR + DR, qoff: qoff + 512],
                                    start=False, stop=True)
                                ex = att_t.tile([P, 512], BF16, tag="ex")
                                nc.scalar.activation(ex[:], ps_sc[:], AF.Exp)
                                if kc >= 4 * qt:
                                    mi = kc - 4 * qt
                                    nc.vector.tensor_mul(ex[:], ex[:], mask_sb[:, mi * 512:(mi + 1) * 512])
                                jb = bb * KB_PER_B + kc
                                nc.tensor.matmul(
                                    ps_o[:], v_sb[:, jb * WKK + hl * DV: jb * WKK + (hl + 1) * DV],
                                    ex[:], start=(kc == 0), stop=(kc == nkc - 1),
                                    skip_group_check=True)
                                nc.tensor.matmul(
                                    ps_d[:], ones_col[:], ex[:],
                                    start=(kc == 0), stop=(kc == nkc - 1),
                                    skip_group_check=True)
                            # normalize: out = ps_o * (1/ps_d) broadcast over partitions
                            ou = att_t.tile([P, 512], F32, tag="ou")
                            nc.scalar.copy(ou[:], ps_o[:])
                            recip = att_t.tile([1, 512], F32, tag="rcp")
                            nc.vector.reciprocal_approx_fast(recip[:], ps_d[:])
                            bc = ps_s_pool.tile([P, 512], F32, tag="pss")
                            nc.tensor.matmul(bc[:], ones_row[:], recip[:], start=True, stop=True)
                            on = att_t.tile([P, 512], BF16, tag="on")
                            nc.vector.tensor_mul(on[:], ou[:], bc[:])
                            blk = bb * QT_PER_B + qt
                            nc.sync.dma_start(
                                a2a_in[blk * (HPC * DV) + hl * DV: blk * (HPC * DV) + (hl + 1) * DV, :],
                                on[:])

            # ================= AllToAll =================
            nc.gpsimd.collective_compute(
                "AllToAll", mybir.AluOpType.bypass,
                replica_groups=[list(range(NCORES))],
                ins=[a2a_in.opt()], outs=[a2a_out.opt()],
            )

            # ================= Phase 3: output projection =================
            with tc.tile_pool(name="p3w", bufs=1) as p3w, \
                 tc.tile_pool(name="p3t", bufs=3) as p3t, \
                 tc.tile_pool(name="ps3", bufs=4, space="PSUM") as ps3:
                wo_sb = p3w.tile([P, H * HID], BF16)
                for r in range(H):
                    nc.sync.dma_start(wo_sb[:, r * HID:(r + 1) * HID],
                                      woT.ap()[r * P:(r + 1) * P, :])
                o_sb = p3w.tile([P, H * TPC], BF16)
                for r in range(H):
                    nc.sync.dma_start(o_sb[:, r * TPC:(r + 1) * TPC],
                                      a2a_out[r * P:(r + 1) * P, :])
                for m in range(NHID):
                    ps = ps3.tile([P, TPC], F32, tag="proj")
                    for r in range(H):
                        nc.tensor.matmul(ps[:], wo_sb[:, r * HID + m * P: r * HID + (m + 1) * P],
                                         o_sb[:, r * TPC:(r + 1) * TPC],
                                         start=(r == 0), stop=(r == H - 1))
                    ot = p3t.tile([P, TPC], F32, tag="ot")
                    nc.scalar.copy(ot[:], ps[:])
                    nc.sync.dma_start(outT.ap()[m * P:(m + 1) * P, :], ot[:])

    nc.finalize()
    return nc


def _bf16(x):
    return np.ascontiguousarray(x.astype(ml_dtypes.bfloat16))


def _rope_tables():
    inv_freq = 1.0 / (THETA ** (np.arange(0, DR, 2, dtype=np.float64) / DR))
    t = np.arange(S, dtype=np.float64)
    freqs = np.outer(t, inv_freq)                       # [S, DR/2]
    emb = np.concatenate((freqs, freqs), axis=-1)       # [S, DR]
    return np.cos(emb).astype(np.float32), np.sin(emb).astype(np.float32)


def prepare_inputs(hidden_states, w_qa, q_a_ln_w, w_qb, w_kva, kv_a_ln_w, w_kvb, w_o):
    hidden_states = np.asarray(hidden_states, dtype=np.float32)
    w_qa = np.asarray(w_qa, dtype=np.float32)
    q_a_ln_w = np.asarray(q_a_ln_w, dtype=np.float32)
    w_qb = np.asarray(w_qb, dtype=np.float32)
    w_kva = np.asarray(w_kva, dtype=np.float32)
    kv_a_ln_w = np.asarray(kv_a_ln_w, dtype=np.float32)
    w_kvb = np.asarray(w_kvb, dtype=np.float32)
    w_o = np.asarray(w_o, dtype=np.float32)

    flat = hidden_states.reshape(T, HID)
    cos, sin = _rope_tables()          # [S, DR]
    scale = DQ ** -0.5

    # cos/sin tables, d-major over global tokens, rows duplicated for 2 heads
    pos = np.arange(T) % S
    cos_d = cos[pos].T                 # [DR, T]
    sin_d = sin[pos].T
    cosq = np.concatenate([cos_d, cos_d], axis=0).astype(np.float32)  # [128, T]
    sinq = np.concatenate([sin_d, sin_d], axis=0).astype(np.float32)

    # diagonal causal masks: mask_p[kp, qf] = 1 if qf >= kp + 128*p
    kp = np.arange(P)[:, None]
    qf = np.arange(512)[None, :]
    mask_list = [(qf >= kp + P * p).astype(np.float32) for p in range(4)]
    masks = _bf16(np.concatenate(mask_list, axis=1))    # [128, 4*512]

    # fold ln weights + scale into up-projection weights
    w_qb_eff = (w_qb * q_a_ln_w[None, :]) * scale       # [H*DQ, QLR]
    w_kvb_eff = w_kvb * kv_a_ln_w[None, :]              # [H*(DN+DV), KVLR]

    wqaT = _bf16(w_qa.T)                                # [HID, QLR]
    wkvaT = _bf16(w_kva.T)                              # [HID, KVLR+DR]
    woT = _bf16(w_o.T)                                  # [H*DV, HID]

    in_maps = []
    for c in range(NCORES):
        heads = [HPC * c + h for h in range(HPC)]
        # per-head-pair w_qb slice, column order [h0 nope | h1 nope | h0 pe | h1 pe]
        rows = []
        for h in heads:
            rows.append(w_qb_eff[h * DQ: h * DQ + DN])          # nope
        for h in heads:
            rows.append(w_qb_eff[h * DQ + DN: h * DQ + DQ])     # pe
        wqbT_c = _bf16(np.concatenate(rows, axis=0).T)          # [QLR, 384]

        krows = [w_kvb_eff[h * (DN + DV): h * (DN + DV) + DN] for h in heads]
        wkvbkT_c = _bf16(np.concatenate(krows, axis=0).T)       # [KVLR, 256]
        vrows = [w_kvb_eff[h * (DN + DV) + DN: (h + 1) * (DN + DV)] for h in heads]
        wkvbvT_c = _bf16(np.concatenate(vrows, axis=0).T)       # [KVLR, 256]

        tok0 = c * TPC
        in_maps.append({
            "hidT": _bf16(flat[tok0:tok0 + TPC].T),             # [HID, TPC]
            "wqaT": wqaT, "wkvaT": wkvaT,
            "wqbT": wqbT_c, "wkvbkT": wkvbkT_c, "wkvbvT": wkvbvT_c,
            "woT": woT,
            "cosq": cosq, "sinq": sinq,
            "cosl": np.ascontiguousarray(cosq[0:DR, tok0:tok0 + TPC]),
            "sinl": np.ascontiguousarray(sinq[0:DR, tok0:tok0 + TPC]),
            "masks": masks,
        })
    return in_maps


def kernel(hidden_states, w_qa, q_a_ln_w, w_qb, w_kva, kv_a_ln_w, w_kvb, w_o,
           _trace=False):
    global _NC_CACHE
    if _NC_CACHE is None:
        _NC_CACHE = build_nc()
    nc = _NC_CACHE
    in_maps = prepare_inputs(hidden_states, w_qa, q_a_ln_w, w_qb, w_kva,
                             kv_a_ln_w, w_kvb, w_o)
    res = run_bass_kernel_spmd(nc, in_maps, core_ids=list(range(NCORES)),
                               trace=_trace)
    out = np.empty((T, HID), dtype=np.float32)
    for c in range(NCORES):
        out[c * TPC:(c + 1) * TPC] = res.results[c]["outT"].T
    if _trace:
        kernel._last_result = res
    return out.reshape(B, S, HID)
